# revision 14
# baseline (speedup 1.0000x reference)
"""Causal attention (B=4, S=2048, D=1024, single head) on 8 TRN2 NeuronCores.

Sharding: data-parallel over batch (4 pairs of cores); within each pair
the K/V context is split by interleaved 128-row chunks (core parity p
owns global k-chunks {2j+p}).

v3 algorithm: scores = q k^T = x (Wq^T Wk) x^T.  The host precomputes
M = Wq^T Wk in fp32 (cast to bf16), so the device projects only
G = x M (the exact cost of the old Q projection) and V = x Wv^T; the
K projection disappears entirely and scores are computed as G x^T
against the x tiles already resident in SBUF.  Each core projects G for
its own 1024 rows; the pair exchanges G halves with a 2-core AllGather
so both cores hold G for all 2048 rows in canonical gathered column
order ([all even 128-blocks | all odd 128-blocks]).  Each core computes
its causal score blocks against its own context and produces
*unnormalized* partial attention output plus per-row partial softmax
denominators (folded into the attn@V loop as 1-column matmuls that
reuse the already-loaded pt weights).  The host adds the two partials
of each pair and normalizes.

v3 schedule notes:
- The exchange dataflow lives entirely on the gpsimd queue (stores,
  collective triggers) so the qhalf stores are never FIFO-blocked
  behind input loads; gt loads for pieces that tiles 0/2 need go on the
  sync queue (free after the x input DMAs).  In v2 the piece-1 store
  sat behind a load that waited on mesh 0 -> 12us PE stall.
- Attention tiles 0,2 depend only on exchange piece 0; 1,3 on piece 1.
- All matmuls run in bf16 (fp32 PSUM accumulation); partial outputs are
  written in bf16 (host accumulates in fp32).
"""

import sys

if "/opt/trn_rl_repo" not in sys.path:
    sys.path.insert(0, "/opt/trn_rl_repo")

import ml_dtypes
import numpy as np

import concourse.bacc as bacc
import concourse.tile as tile
from concourse import mybir
from concourse.bass_utils import run_bass_kernel_spmd

# bass_utils imports antenv.axon_hooks when tracing is requested; the image's
# antenv lacks that module, so provide a no-op fallback rather than crashing.
try:
    import antenv.axon_hooks  # noqa: F401
except ImportError:
    import types as _types

    _ah = _types.ModuleType("antenv.axon_hooks")
    _ah._hook = None
    _ah.set_axon_ntff_profile_hook = lambda h: setattr(_ah, "_hook", h)
    _ah.get_axon_ntff_profile_hook = lambda: _ah._hook
    sys.modules["antenv.axon_hooks"] = _ah

B, S, D = 4, 2048, 1024
NB = S // 128          # 16 q-blocks of 128 per batch
NT = S // 512          # 4 q-tiles of 512
IC = D // 128          # 8 contraction chunks
OC = D // 128          # 8 output-dim chunks
LC = 8                 # local k-chunks per core (S/2/128)
NMSK = 16              # mask tiles: 4 per q-tile
SCALE = 1.0 / np.sqrt(D)  # 0.03125
NJ_TILE = [4, 8, 4, 8]  # local k-chunks needed per gathered q-tile
NWARM = 4              # PE warm-up matmuls
NCCWARM = 6            # CC warm-up collectives (keep the CC core hot)

BF16 = mybir.dt.bfloat16
F32 = mybir.dt.float32

_module_cache = None
last_results = None  # BassKernelResults of the most recent run (for test harness)


def _masked_js(tt):
    """Local chunk indices whose score blocks need a mask for q-tile tt."""
    return range(4) if tt in (0, 2) else range(4, 8)


def _build_module():
    nc = bacc.Bacc("TRN2", target_bir_lowering=False, debug=False, num_devices=8)
    # All inputs are packed partition-major on the host so every input DMA
    # moves multi-KB contiguous runs on both the DRAM and SBUF side (small
    # strided lines are descriptor-overhead-bound at ~70-120 GB/s).
    # xT is half-major: [h, p, i, c]; m4 is M = Wq^T Wk packed as 4
    # o-group slabs [g, p, i, 256] exactly like the old wq packing.
    xT = nc.dram_tensor("xT", [2, 128, IC, 512], BF16, kind="ExternalInput").ap()
    m4 = nc.dram_tensor("m4", [4, 128, IC, 256], BF16, kind="ExternalInput").ap()
    wvT = nc.dram_tensor("wvT", [128, IC, 1024], BF16, kind="ExternalInput").ap()
    msk = nc.dram_tensor("msk", [128, NMSK, 512], BF16, kind="ExternalInput").ap()
    out_p = nc.dram_tensor("out_p", [S, D], BF16, kind="ExternalOutput").ap()
    den_out = nc.dram_tensor("den_out", [128, NB], F32, kind="ExternalOutput").ap()

    with tile.TileContext(nc) as tc:
        with (
            tc.tile_pool(name="wp", bufs=1) as wp,
            tc.tile_pool(name="xp", bufs=1) as xp,
            tc.tile_pool(name="kqv", bufs=1) as kqv,
            tc.tile_pool(name="mp", bufs=1) as mp,
            tc.tile_pool(name="ptp", bufs=2) as ptp,
            tc.tile_pool(name="stg", bufs=4) as stg,
            tc.tile_pool(name="qsg", bufs=2) as qsg,
            tc.tile_pool(name="dr", bufs=1, space="DRAM") as dr,
        ):
            # ---- PE warm-up: garbage matmuls on zeroed tiles, issued
            #      before any input-dependent work so the PE leaves its
            #      cold clock state while input DMAs are in flight ----
            warm_w = mp.tile([128, 128], BF16, tag="warmw", name="warmw")
            warm_x = mp.tile([128, 512], BF16, tag="warmx", name="warmx")
            nc.gpsimd.memset(warm_w, 0.0)
            nc.gpsimd.memset(warm_x, 0.0)
            with tc.tile_pool(name="psw", bufs=1, space="PSUM") as psw:
                wpp = psw.tile([128, 512], F32, tag="warm", bufs=1, name="warmp")
                for _ in range(NWARM):
                    nc.tensor.matmul(wpp, lhsT=warm_w, rhs=warm_x, start=True, stop=True)

            xt_all = xp.tile([128, 2, IC, 512], BF16, tag="xt", name="xt")
            m_all = wp.tile([128, 4, IC, 256], BF16, tag="m4", name="m4")
            wv_all = wp.tile([128, IC, 1024], BF16, tag="wv", name="wv")

            # Queues are FIFO, so order input pieces by when the PE needs
            # them.  sync carries x only (then gt loads + output DMAs);
            # scalar carries M slabs, wv, masks; gpsimd carries the
            # exchange stores + collective triggers.
            nc.sync.dma_start(xt_all[:, 0, 0:4, :], xT[0, :, 0:4, :])
            nc.sync.dma_start(xt_all[:, 0, 4:8, :], xT[0, :, 4:8, :])
            for g in range(4):
                nc.scalar.dma_start(m_all[:, g, 0:4, :], m4[g, :, 0:4, :])
            for g in range(4):
                nc.scalar.dma_start(m_all[:, g, 4:8, :], m4[g, :, 4:8, :])
            # x half 1 rides scalar after the m4 slabs: during the first
            # ~10us both queues stream and the G-st0 working set (x h0 +
            # all of m4) is exactly HBM-bound, so x h1 must not steal
            # bandwidth from the m4 slabs
            nc.scalar.dma_start(xt_all[:, 1, :, :], xT[1])
            nc.scalar.dma_start(wv_all, wvT)
            mask_all = mp.tile([128, NMSK, 512], BF16, tag="masks", name="masks")
            nc.scalar.dma_start(mask_all, msk)
            ones_sb = mp.tile([128, 1], BF16, tag="ones", name="ones")
            nc.any.memset(ones_sb, 1.0)

            gt_all = kqv.tile([128, IC, S], BF16, tag="gt", name="gt")
            vn_sb = [kqv.tile([128, D], BF16, tag=f"vn{j}", name=f"vn{j}") for j in range(LC)]
            den_sb = mp.tile([128, NB], F32, tag="den", name="den")

            # DRAM bounce buffers for the pairwise G-half exchange (split in
            # two st-pieces so the exchange pipelines with the projections)
            qhalf = [dr.tile([128, IC * 512], BF16, name=f"qhalf{st}") for st in range(2)]
            qfull = [dr.tile([2 * 128, IC * 512], BF16, name=f"qfull{st}") for st in range(2)]

            # ---- CC warm-up: a chain of small pair-collectives keeps the
            #      CC core busy through the input-fill phase.  A cold CC
            #      pays ~11us from trigger to mesh-begin; back-to-back
            #      meshes start within ~2us, so the real exchange begins
            #      almost immediately after its trigger. ----
            ccw_in = dr.tile([128, 512], BF16, name="ccw_in")
            ccw_out = [
                dr.tile([256, 512], BF16, name=f"ccw_out{d}") for d in range(NCCWARM)
            ]
            nc.sync.dma_start(ccw_in, warm_x)
            for d in range(NCCWARM):
                nc.gpsimd.collective_compute(
                    kind="AllGather",
                    op=mybir.AluOpType.bypass,
                    replica_groups=[[0, 1], [2, 3], [4, 5], [6, 7]],
                    ins=[ccw_in],
                    outs=[ccw_out[d]],
                )

            def xs(i, h):
                return xt_all[:, h, i, :]

            def proj_iouter(ps1, lhs_slices, rhs_slices, dsts, pname, schedule=None):
                # schedule: list of (i_range, o_list) sub-groups; flags stay
                # start=(i==0)/stop=(i==IC-1) so any i order that runs i==0
                # first and i==IC-1 last per psum is legal
                schedule = schedule or [(range(IC), range(len(dsts)))]
                pps = [
                    ps1.tile([128, 512], F32, tag=f"proj8_{o}", bufs=1, name=f"{pname}{o}")
                    for o in range(len(dsts))
                ]
                for irng, orng in schedule:
                    for i in irng:
                        for o in orng:
                            nc.tensor.matmul(
                                pps[o],
                                lhsT=lhs_slices(i, o),
                                rhs=rhs_slices(i, o),
                                start=(i == 0),
                                stop=(i == IC - 1),
                            )
                for o, dst in enumerate(dsts):
                    dst(pps[o])

            def copy_to(dst):
                return lambda pp: nc.vector.tensor_copy(dst, pp)

            def g_own_phase(ps1, st, schedule=None):
                """Project this core's own G half (G = x M, own rows),
                stage to SBUF, then one batched DMA into qhalf[st] issued
                on the otherwise-idle gpsimd queue."""
                qs = qsg.tile([128, IC, 512], BF16, tag="qsg", name="qsg")
                proj_iouter(
                    ps1,
                    lambda i, o: m_all[:, o // 2, i, 128 * (o % 2) : 128 * (o % 2 + 1)],
                    lambda i, o: xs(i, st),
                    [copy_to(qs[:, o, :]) for o in range(IC)],
                    f"pg{st}",
                    schedule=schedule,
                )
                # store via sync (hardware DGE): gpsimd SWDGE stores break
                # the collective trigger handshake (CC core sleeps ~70us)
                nc.sync.dma_start(
                    qhalf[st].rearrange("p (o c) -> p o c", o=IC), qs
                )
                nc.gpsimd.collective_compute(
                    kind="AllGather",
                    op=mybir.AluOpType.bypass,
                    replica_groups=[[0, 1], [2, 3], [4, 5], [6, 7]],
                    ins=[qhalf[st]],
                    outs=[qfull[st]],
                )

            def gt_load(st, r, eng, split=False):
                # rank r's piece st lands at gathered columns
                # [1024 r + 512 st, 1024 r + 512 (st+1)) == q-tile 2r+st
                c0 = 1024 * r + 512 * st
                src = qfull[st][128 * r : 128 * (r + 1), :].rearrange(
                    "p (o c) -> p o c", o=IC
                )
                # split the latency-critical loads so the first score
                # matmuls (i-chunks 0-3) can start half a load earlier
                ranges = [(0, 4), (4, 8)] if split else [(0, 8)]
                for a, b in ranges:
                    eng.dma_start(
                        gt_all[:, a:b, c0 : c0 + 512], src[:, a:b, :]
                    )

            with tc.tile_pool(name="ps1", bufs=1, space="PSUM") as ps1:
                # The G-st0 working set (x h0 + all of m4, 3MB over 13.8us)
                # is near the HBM roofline; run every slab's i0-3 first
                # (needs only x0a + the m4 a-halves) then i4-7 so demand
                # tracks the DMA arrival order with no burst over ~220GB/s
                g_own_phase(
                    ps1,
                    0,
                    schedule=[
                        (range(0, 4), [2 * g, 2 * g + 1]) for g in range(4)
                    ]
                    + [(range(4, 8), [2 * g, 2 * g + 1]) for g in range(4)],
                )
                g_own_phase(ps1, 1)
                # gt loads spread over four queues so the two pieces of an
                # exchange transfer in parallel and nothing hot is blocked:
                # tile 0's load on sync (idle after the x inputs), tile 2's
                # on scalar (its exps transitively depend on it anyway),
                # tiles 1/3 on gpsimd after the triggers.
                gt_load(0, 0, nc.sync, split=True)
                gt_load(0, 1, nc.gpsimd)
                gt_load(1, 0, nc.gpsimd, split=True)
                gt_load(1, 1, nc.gpsimd)

                # V projection: all 8 local chunks, two 8-psum rounds
                for half in range(2):
                    proj_iouter(
                        ps1,
                        lambda i, c, _h=half: xt_all[
                            :, _h, i, 128 * (c // 2) : 128 * (c // 2 + 1)
                        ],
                        lambda i, c: wv_all[:, i, 512 * (c % 2) : 512 * (c % 2 + 1)],
                        [
                            (
                                lambda dst, eng: lambda pp: eng(dst, pp)
                            )(
                                vn_sb[4 * half + c // 2][
                                    :, 512 * (c % 2) : 512 * (c % 2 + 1)
                                ],
                                nc.vector.tensor_copy if c % 2 == 0 else nc.scalar.copy,
                            )
                            for c in range(8)
                        ],
                        f"pv{half}",
                    )

            # ---- phase 2: attention over gathered q-tiles; tiles 0 and 2
            #      only depend on the first exchange piece ----
            with tc.tile_pool(name="ps2", bufs=2, space="PSUM") as ps:

                def attention_tile(tt):
                    nj = NJ_TILE[tt]
                    masked = set(_masked_js(tt))
                    pt_tiles = []
                    for j in range(nj):
                        # in a masked (diagonal-region) block, the first
                        # 128*(j%4) gathered q-columns are fully masked out —
                        # skip computing them entirely
                        off = 128 * (j % 4) if j in masked else 0
                        sp = ps.tile([128, 512], F32, tag="score", bufs=4, name="score")
                        for i in range(IC):
                            nc.tensor.matmul(
                                sp[:, off:512],
                                lhsT=xt_all[:, j // 4, i, 128 * (j % 4) : 128 * (j % 4 + 1)],
                                rhs=gt_all[:, i, 512 * tt + off : 512 * (tt + 1)],
                                start=(i == 0),
                                stop=(i == IC - 1),
                            )
                        pt = ptp.tile([128, 512], BF16, tag=f"pt{j}", name=f"pt{j}")
                        nc.scalar.activation(
                            pt[:, off:512],
                            sp[:, off:512],
                            mybir.ActivationFunctionType.Exp,
                            scale=SCALE,
                        )
                        if j in masked:
                            m = 4 * tt + (j % 4)
                            nc.vector.tensor_mul(
                                pt[:, off:512], pt[:, off:512], mask_all[:, m, off:512]
                            )
                        pt_tiles.append(pt)

                    for qq in (3, 2, 1, 0):
                        qbg = 4 * tt + qq        # gathered q-block index
                        njs = (qbg % 8) + 1      # causal chunk count in gathered order
                        ost = stg.tile([128, D], BF16, tag="ost", name="ost")
                        ap0 = ps.tile([128, 512], F32, tag="attn", bufs=3, name="attn")
                        ap1 = ps.tile([128, 512], F32, tag="attn", bufs=3, name="attn")
                        dps = ps.tile([128, 1], F32, tag="denp", bufs=1, name="denp")
                        for j in range(njs):
                            lhs = pt_tiles[j][:, 128 * qq : 128 * (qq + 1)]
                            st_, sp_ = (j == 0), (j == njs - 1)
                            nc.tensor.matmul(
                                ap0, lhsT=lhs, rhs=vn_sb[j][:, 0:512], start=st_, stop=sp_
                            )
                            nc.tensor.matmul(
                                ap1, lhsT=lhs, rhs=vn_sb[j][:, 512:1024], start=st_, stop=sp_
                            )
                            # softmax denominator rides along: 1-column
                            # matmul reusing the pt weights just loaded
                            nc.tensor.matmul(
                                dps, lhsT=lhs, rhs=ones_sb, start=st_, stop=sp_
                            )
                        nc.vector.tensor_copy(den_sb[:, qbg : qbg + 1], dps)
                        # alternate copy engines and DMA each half as soon
                        # as its copy lands
                        nc.vector.tensor_copy(ost[:, 0:512], ap0)
                        nc.sync.dma_start(
                            out_p[128 * qbg : 128 * (qbg + 1), 0:512], ost[:, 0:512]
                        )
                        nc.scalar.copy(ost[:, 512:1024], ap1)
                        nc.sync.dma_start(
                            out_p[128 * qbg : 128 * (qbg + 1), 512:1024],
                            ost[:, 512:1024],
                        )

                attention_tile(0)
                attention_tile(2)
                attention_tile(1)
                attention_tile(3)
                nc.sync.dma_start(den_out, den_sb)

    nc.compile()
    return nc


def _get_module():
    global _module_cache
    if _module_cache is None:
        _module_cache = _build_module()
    return _module_cache


def _gathered_q(p):
    """Global q index for gathered position p (vectorized)."""
    p = np.asarray(p)
    blk = p // 128
    even = blk < 8
    gb = np.where(even, 2 * blk, 2 * (blk - 8) + 1)
    return 128 * gb + p % 128


def _host_masks(par: int) -> np.ndarray:
    """[NMSK*128, 512] bf16 causal masks in gathered q order."""
    out = np.zeros((NMSK * 128, 512), dtype=np.float32)
    k = np.arange(128)[:, None]
    ql = np.arange(512)[None, :]
    for tt in range(NT):
        for idx, j in enumerate(_masked_js(tt)):
            m = 4 * tt + idx
            g = 2 * j + par  # global k-chunk of local chunk j
            q_global = _gathered_q(512 * tt + ql)
            out[128 * m : 128 * (m + 1), :] = (q_global >= 128 * g + k).astype(
                np.float32
            )
    return out.astype(ml_dtypes.bfloat16)


def kernel(x, Wq, Wk, Wv, _trace=False):
    global last_results
    nc = _get_module()

    bf = ml_dtypes.bfloat16

    # M = Wq^T Wk in fp32 (scores = x M x^T), packed like the old wq:
    # 4 o-group slabs [g, p, i, 256], partition-major
    M = Wq.T.astype(np.float32) @ Wk.astype(np.float32)
    m4 = np.ascontiguousarray(
        M.astype(bf).reshape(IC, 128, 4, 256).transpose(2, 1, 0, 3)
    )
    wvT = np.ascontiguousarray(Wv.T.astype(bf).reshape(IC, 128, D).transpose(1, 0, 2))
    masks = [
        np.ascontiguousarray(
            _host_masks(par).reshape(NMSK, 128, 512).transpose(1, 0, 2)
        )
        for par in range(2)
    ]

    # per-parity column selection: core owns global k-chunks {2j+par}
    own_cols = [
        (128 * (2 * np.arange(LC)[:, None] + par) + np.arange(128)[None, :]).reshape(-1)
        for par in range(2)
    ]

    in_maps = []
    for c in range(8):
        b, par = c // 2, c % 2
        xTb = x[b].T[:, own_cols[par]].astype(bf)  # [D, S//2]
        # pack partition-major: [h, p, i, c]
        xpk = np.ascontiguousarray(
            xTb.reshape(IC, 128, 2, 512).transpose(2, 1, 0, 3)
        )
        in_maps.append(
            {
                "xT": xpk,
                "m4": m4,
                "wvT": wvT,
                "msk": masks[par],
            }
        )

    kwargs = {}
    if _trace:
        kwargs["trace"] = True
    res = run_bass_kernel_spmd(nc, in_maps, core_ids=list(range(8)), **kwargs)
    last_results = res

    # rows come back in gathered order; gath_row[q] = gathered position of q
    gath_row = np.empty(S, dtype=np.int64)
    gath_row[_gathered_q(np.arange(S))] = np.arange(S)

    out = np.empty((B, S, D), dtype=np.float32)
    for b in range(B):
        rA = res.results[2 * b]
        rB = res.results[2 * b + 1]
        num = rA["out_p"].astype(np.float32) + rB["out_p"].astype(np.float32)
        # den_out[r, qbg] is the partial denominator of gathered row
        # 128*qbg + r
        den = (
            rA["den_out"].T.reshape(S) + rB["den_out"].T.reshape(S)
        )
        out[b] = (num / den[:, None])[gath_row]
    return out


# revision 20
# speedup vs baseline: 1.3689x; 1.3689x over previous
"""Causal attention (B=4, S=2048, D=1024, single head) on 8 TRN2 NeuronCores.

Sharding: data-parallel over batch (4 pairs of cores); within each pair
the K/V context is split by interleaved 128-row chunks (core parity p
owns global k-chunks {2j+p}).

v3 algorithm: scores = q k^T = x (Wq^T Wk) x^T.  The host precomputes
M = Wq^T Wk in fp32 (cast to bf16), so the device projects only
G = x M (the exact cost of the old Q projection) and V = x Wv^T; the
K projection disappears entirely and scores are computed as G x^T
against the x tiles already resident in SBUF.  Each core projects G for
its own 1024 rows; the pair exchanges G halves with a 2-core AllGather
so both cores hold G for all 2048 rows in canonical gathered column
order ([all even 128-blocks | all odd 128-blocks]).  Each core computes
its causal score blocks against its own context and produces
*unnormalized* partial attention output plus per-row partial softmax
denominators (folded into the attn@V loop as 1-column matmuls that
reuse the already-loaded pt weights).  The host adds the two partials
of each pair and normalizes.

v3 schedule notes:
- The exchange dataflow lives entirely on the gpsimd queue (stores,
  collective triggers) so the qhalf stores are never FIFO-blocked
  behind input loads; gt loads for pieces that tiles 0/2 need go on the
  sync queue (free after the x input DMAs).  In v2 the piece-1 store
  sat behind a load that waited on mesh 0 -> 12us PE stall.
- Attention tiles 0,2 depend only on exchange piece 0; 1,3 on piece 1.
- All matmuls run in bf16 (fp32 PSUM accumulation); partial outputs are
  written in bf16 (host accumulates in fp32).
"""

import sys

if "/opt/trn_rl_repo" not in sys.path:
    sys.path.insert(0, "/opt/trn_rl_repo")

import ml_dtypes
import numpy as np

import concourse.bacc as bacc
import concourse.tile as tile
from concourse import mybir
from concourse.bass_utils import run_bass_kernel_spmd

# bass_utils imports antenv.axon_hooks when tracing is requested; the image's
# antenv lacks that module, so provide a no-op fallback rather than crashing.
try:
    import antenv.axon_hooks  # noqa: F401
except ImportError:
    import types as _types

    _ah = _types.ModuleType("antenv.axon_hooks")
    _ah._hook = None
    _ah.set_axon_ntff_profile_hook = lambda h: setattr(_ah, "_hook", h)
    _ah.get_axon_ntff_profile_hook = lambda: _ah._hook
    sys.modules["antenv.axon_hooks"] = _ah

B, S, D = 4, 2048, 1024
NB = S // 128          # 16 q-blocks of 128 per batch
NT = S // 512          # 4 q-tiles of 512
IC = D // 128          # 8 contraction chunks
OC = D // 128          # 8 output-dim chunks
LC = 8                 # local k-chunks per core (S/2/128)
NMSK = 16              # mask tiles: 4 per q-tile
SCALE = 1.0 / np.sqrt(D)  # 0.03125
NJ_TILE = [4, 8, 4, 8]  # local k-chunks needed per gathered q-tile
NWARM = 4              # PE warm-up matmuls

BF16 = mybir.dt.bfloat16
F32 = mybir.dt.float32

_module_cache = None
last_results = None  # BassKernelResults of the most recent run (for test harness)


def _masked_js(tt):
    """Local chunk indices whose score blocks need a mask for q-tile tt."""
    return range(4) if tt in (0, 2) else range(4, 8)


def _build_module():
    nc = bacc.Bacc("TRN2", target_bir_lowering=False, debug=False, num_devices=8)
    # All inputs are packed partition-major on the host so every input DMA
    # moves multi-KB contiguous runs on both the DRAM and SBUF side (small
    # strided lines are descriptor-overhead-bound at ~70-120 GB/s).
    # xT is half-major: [h, p, i, c]; m4 is M = Wq^T Wk packed as 4
    # o-group slabs [g, p, i, 256] exactly like the old wq packing.
    xT = nc.dram_tensor("xT", [2, 128, IC, 512], BF16, kind="ExternalInput").ap()
    m4 = nc.dram_tensor("m4", [4, 128, IC, 256], BF16, kind="ExternalInput").ap()
    wvT = nc.dram_tensor("wvT", [128, IC, 1024], BF16, kind="ExternalInput").ap()
    msk = nc.dram_tensor("msk", [128, NMSK, 512], BF16, kind="ExternalInput").ap()
    out_p = nc.dram_tensor("out_p", [S, D], BF16, kind="ExternalOutput").ap()
    den_out = nc.dram_tensor("den_out", [128, NB], F32, kind="ExternalOutput").ap()

    with tile.TileContext(nc) as tc:
        with (
            tc.tile_pool(name="wp", bufs=1) as wp,
            tc.tile_pool(name="xp", bufs=1) as xp,
            tc.tile_pool(name="kqv", bufs=1) as kqv,
            tc.tile_pool(name="mp", bufs=1) as mp,
            tc.tile_pool(name="ptp", bufs=2) as ptp,
            tc.tile_pool(name="stg", bufs=4) as stg,
            tc.tile_pool(name="qsg", bufs=2) as qsg,
            tc.tile_pool(name="dr", bufs=1, space="DRAM") as dr,
        ):
            # ---- PE warm-up: garbage matmuls on zeroed tiles, issued
            #      before any input-dependent work so the PE leaves its
            #      cold clock state while input DMAs are in flight ----
            warm_w = mp.tile([128, 128], BF16, tag="warmw", name="warmw")
            warm_x = mp.tile([128, 512], BF16, tag="warmx", name="warmx")
            nc.gpsimd.memset(warm_w, 0.0)
            nc.gpsimd.memset(warm_x, 0.0)
            with tc.tile_pool(name="psw", bufs=1, space="PSUM") as psw:
                wpp = psw.tile([128, 512], F32, tag="warm", bufs=1, name="warmp")
                for _ in range(NWARM):
                    nc.tensor.matmul(wpp, lhsT=warm_w, rhs=warm_x, start=True, stop=True)

            xt_all = xp.tile([128, 2, IC, 512], BF16, tag="xt", name="xt")
            m_all = wp.tile([128, 4, IC, 256], BF16, tag="m4", name="m4")
            wv_all = wp.tile([128, IC, 1024], BF16, tag="wv", name="wv")

            # Queues are FIFO, so order input pieces by when the PE needs
            # them.  sync carries x only (then gt loads + output DMAs);
            # scalar carries M slabs, wv, masks; gpsimd carries the
            # exchange stores + collective triggers.
            nc.sync.dma_start(xt_all[:, 0, 0:4, :], xT[0, :, 0:4, :])
            nc.sync.dma_start(xt_all[:, 0, 4:8, :], xT[0, :, 4:8, :])
            for g in range(4):
                nc.scalar.dma_start(m_all[:, g, 0:4, :], m4[g, :, 0:4, :])
            for g in range(4):
                nc.scalar.dma_start(m_all[:, g, 4:8, :], m4[g, :, 4:8, :])
            # x half 1 rides scalar after the m4 slabs: during the first
            # ~10us both queues stream and the G-st0 working set (x h0 +
            # all of m4) is exactly HBM-bound, so x h1 must not steal
            # bandwidth from the m4 slabs
            nc.scalar.dma_start(xt_all[:, 1, :, :], xT[1])
            nc.scalar.dma_start(wv_all, wvT)
            mask_all = mp.tile([128, NMSK, 512], BF16, tag="masks", name="masks")
            nc.scalar.dma_start(mask_all, msk)
            ones_sb = mp.tile([128, 1], BF16, tag="ones", name="ones")
            nc.any.memset(ones_sb, 1.0)

            gt_all = kqv.tile([128, IC, S], BF16, tag="gt", name="gt")
            vn_sb = [kqv.tile([128, D], BF16, tag=f"vn{j}", name=f"vn{j}") for j in range(LC)]
            den_sb = mp.tile([128, NB], F32, tag="den", name="den")

            # DRAM bounce buffers for the pairwise G-half exchange (split in
            # two st-pieces so the exchange pipelines with the projections).
            # Note: all collectives serialize behind the framework's kernel
            # entry barrier (~40us) plus ~12us first-trigger latency, so
            # mesh 0 cannot begin before ~52us no matter how early its
            # inputs are staged; warm-up dummy collectives only queue in
            # front of the real meshes and make things worse.
            qhalf = [dr.tile([128, IC * 512], BF16, name=f"qhalf{st}") for st in range(2)]
            qfull = [dr.tile([2 * 128, IC * 512], BF16, name=f"qfull{st}") for st in range(2)]

            def xs(i, h):
                return xt_all[:, h, i, :]

            def proj_iouter(ps1, lhs_slices, rhs_slices, dsts, pname, schedule=None):
                # schedule: list of (i_range, o_list) sub-groups; flags stay
                # start=(i==0)/stop=(i==IC-1) so any i order that runs i==0
                # first and i==IC-1 last per psum is legal
                schedule = schedule or [(range(IC), range(len(dsts)))]
                pps = [
                    ps1.tile([128, 512], F32, tag=f"proj8_{o}", bufs=1, name=f"{pname}{o}")
                    for o in range(len(dsts))
                ]
                for irng, orng in schedule:
                    for i in irng:
                        for o in orng:
                            nc.tensor.matmul(
                                pps[o],
                                lhsT=lhs_slices(i, o),
                                rhs=rhs_slices(i, o),
                                start=(i == 0),
                                stop=(i == IC - 1),
                            )
                for o, dst in enumerate(dsts):
                    dst(pps[o])

            def copy_to(dst):
                return lambda pp: nc.vector.tensor_copy(dst, pp)

            def g_own_phase(ps1, st, schedule=None):
                """Project this core's own G half (G = x M, own rows),
                stage to SBUF, then one batched DMA into qhalf[st] issued
                on the otherwise-idle gpsimd queue."""
                qs = qsg.tile([128, IC, 512], BF16, tag="qsg", name="qsg")
                proj_iouter(
                    ps1,
                    lambda i, o: m_all[:, o // 2, i, 128 * (o % 2) : 128 * (o % 2 + 1)],
                    lambda i, o: xs(i, st),
                    [copy_to(qs[:, o, :]) for o in range(IC)],
                    f"pg{st}",
                    schedule=schedule,
                )
                # store via sync (hardware DGE): gpsimd SWDGE stores break
                # the collective trigger handshake (CC core sleeps ~70us)
                nc.sync.dma_start(
                    qhalf[st].rearrange("p (o c) -> p o c", o=IC), qs
                )
                nc.gpsimd.collective_compute(
                    kind="AllGather",
                    op=mybir.AluOpType.bypass,
                    replica_groups=[[0, 1], [2, 3], [4, 5], [6, 7]],
                    ins=[qhalf[st]],
                    outs=[qfull[st]],
                )

            def gt_load(st, r, eng, split=False):
                # rank r's piece st lands at gathered columns
                # [1024 r + 512 st, 1024 r + 512 (st+1)) == q-tile 2r+st
                c0 = 1024 * r + 512 * st
                src = qfull[st][128 * r : 128 * (r + 1), :].rearrange(
                    "p (o c) -> p o c", o=IC
                )
                # split the latency-critical loads so the first score
                # matmuls (i-chunks 0-3) can start half a load earlier
                ranges = [(0, 4), (4, 8)] if split else [(0, 8)]
                for a, b in ranges:
                    eng.dma_start(
                        gt_all[:, a:b, c0 : c0 + 512], src[:, a:b, :]
                    )

            with tc.tile_pool(name="ps1", bufs=1, space="PSUM") as ps1:
                # The G-st0 working set (x h0 + all of m4, 3MB over 13.8us)
                # is near the HBM roofline; run every slab's i0-3 first
                # (needs only x0a + the m4 a-halves) then i4-7 so demand
                # tracks the DMA arrival order with no burst over ~220GB/s
                g_own_phase(
                    ps1,
                    0,
                    schedule=[
                        (range(0, 4), [2 * g, 2 * g + 1]) for g in range(4)
                    ]
                    + [(range(4, 8), [2 * g, 2 * g + 1]) for g in range(4)],
                )
                g_own_phase(ps1, 1)
                # gt loads spread over four queues so the two pieces of an
                # exchange transfer in parallel and nothing hot is blocked:
                # tile 0's load on sync (idle after the x inputs), tile 2's
                # on scalar (its exps transitively depend on it anyway),
                # tiles 1/3 on gpsimd after the triggers.
                # tile 0's load goes alone on sync so it owns the HBM
                # window right after mesh 0 (mesh 1's copy phase runs
                # then); tile 2's load is issued later, from inside
                # attention tile 0 (see attention_tile), tiles 1/3 on
                # gpsimd after the triggers.
                gt_load(0, 0, nc.sync, split=True)
                gt_load(1, 0, nc.gpsimd, split=True)
                gt_load(1, 1, nc.gpsimd)

                # V projection: all 8 local chunks, two 8-psum rounds
                for half in range(2):
                    proj_iouter(
                        ps1,
                        lambda i, c, _h=half: xt_all[
                            :, _h, i, 128 * (c // 2) : 128 * (c // 2 + 1)
                        ],
                        lambda i, c: wv_all[:, i, 512 * (c % 2) : 512 * (c % 2 + 1)],
                        [
                            (
                                lambda dst, eng: lambda pp: eng(dst, pp)
                            )(
                                vn_sb[4 * half + c // 2][
                                    :, 512 * (c % 2) : 512 * (c % 2 + 1)
                                ],
                                nc.vector.tensor_copy if c % 2 == 0 else nc.scalar.copy,
                            )
                            for c in range(8)
                        ],
                        f"pv{half}",
                    )

            # ---- phase 2: attention over gathered q-tiles; tiles 0 and 2
            #      only depend on the first exchange piece ----
            with tc.tile_pool(name="ps2", bufs=2, space="PSUM") as ps:

                def attention_tile(tt, after_exps=None):
                    nj = NJ_TILE[tt]
                    masked = set(_masked_js(tt))
                    pt_tiles = []
                    for j in range(nj):
                        # in a masked (diagonal-region) block, the first
                        # 128*(j%4) gathered q-columns are fully masked out —
                        # skip computing them entirely
                        off = 128 * (j % 4) if j in masked else 0
                        sp = ps.tile([128, 512], F32, tag="score", bufs=4, name="score")
                        for i in range(IC):
                            nc.tensor.matmul(
                                sp[:, off:512],
                                lhsT=xt_all[:, j // 4, i, 128 * (j % 4) : 128 * (j % 4 + 1)],
                                rhs=gt_all[:, i, 512 * tt + off : 512 * (tt + 1)],
                                start=(i == 0),
                                stop=(i == IC - 1),
                            )
                        pt = ptp.tile([128, 512], BF16, tag=f"pt{j}", name=f"pt{j}")
                        nc.scalar.activation(
                            pt[:, off:512],
                            sp[:, off:512],
                            mybir.ActivationFunctionType.Exp,
                            scale=SCALE,
                        )
                        if j in masked:
                            m = 4 * tt + (j % 4)
                            nc.vector.tensor_mul(
                                pt[:, off:512], pt[:, off:512], mask_all[:, m, off:512]
                            )
                        pt_tiles.append(pt)

                    if after_exps is not None:
                        after_exps()

                    for qq in (3, 2, 1, 0):
                        qbg = 4 * tt + qq        # gathered q-block index
                        njs = (qbg % 8) + 1      # causal chunk count in gathered order
                        ost = stg.tile([128, D], BF16, tag="ost", name="ost")
                        ap0 = ps.tile([128, 512], F32, tag="attn", bufs=3, name="attn")
                        ap1 = ps.tile([128, 512], F32, tag="attn", bufs=3, name="attn")
                        dps = ps.tile([128, 1], F32, tag="denp", bufs=1, name="denp")
                        for j in range(njs):
                            lhs = pt_tiles[j][:, 128 * qq : 128 * (qq + 1)]
                            st_, sp_ = (j == 0), (j == njs - 1)
                            nc.tensor.matmul(
                                ap0, lhsT=lhs, rhs=vn_sb[j][:, 0:512], start=st_, stop=sp_
                            )
                            nc.tensor.matmul(
                                ap1, lhsT=lhs, rhs=vn_sb[j][:, 512:1024], start=st_, stop=sp_
                            )
                            # softmax denominator rides along: 1-column
                            # matmul reusing the pt weights just loaded
                            nc.tensor.matmul(
                                dps, lhsT=lhs, rhs=ones_sb, start=st_, stop=sp_
                            )
                        nc.vector.tensor_copy(den_sb[:, qbg : qbg + 1], dps)
                        # alternate copy engines and DMA each half as soon
                        # as its copy lands
                        nc.vector.tensor_copy(ost[:, 0:512], ap0)
                        nc.sync.dma_start(
                            out_p[128 * qbg : 128 * (qbg + 1), 0:512], ost[:, 0:512]
                        )
                        nc.scalar.copy(ost[:, 512:1024], ap1)
                        nc.sync.dma_start(
                            out_p[128 * qbg : 128 * (qbg + 1), 512:1024],
                            ost[:, 512:1024],
                        )

                # tile 2's gt load issues from the scalar queue after tile
                # 0's exps: by then the gt(0,0) load and mesh 1's copy
                # phase are done fighting for HBM, and tile 2 only needs
                # the data ~8us later
                attention_tile(0, after_exps=lambda: gt_load(0, 1, nc.scalar))
                attention_tile(2)
                attention_tile(1)
                attention_tile(3)
                nc.sync.dma_start(den_out, den_sb)

    nc.compile()
    return nc


def _get_module():
    global _module_cache
    if _module_cache is None:
        _module_cache = _build_module()
    return _module_cache


def _gathered_q(p):
    """Global q index for gathered position p (vectorized)."""
    p = np.asarray(p)
    blk = p // 128
    even = blk < 8
    gb = np.where(even, 2 * blk, 2 * (blk - 8) + 1)
    return 128 * gb + p % 128


def _host_masks(par: int) -> np.ndarray:
    """[NMSK*128, 512] bf16 causal masks in gathered q order."""
    out = np.zeros((NMSK * 128, 512), dtype=np.float32)
    k = np.arange(128)[:, None]
    ql = np.arange(512)[None, :]
    for tt in range(NT):
        for idx, j in enumerate(_masked_js(tt)):
            m = 4 * tt + idx
            g = 2 * j + par  # global k-chunk of local chunk j
            q_global = _gathered_q(512 * tt + ql)
            out[128 * m : 128 * (m + 1), :] = (q_global >= 128 * g + k).astype(
                np.float32
            )
    return out.astype(ml_dtypes.bfloat16)


def kernel(x, Wq, Wk, Wv, _trace=False):
    global last_results
    nc = _get_module()

    bf = ml_dtypes.bfloat16

    # M = Wq^T Wk in fp32 (scores = x M x^T), packed like the old wq:
    # 4 o-group slabs [g, p, i, 256], partition-major
    M = Wq.T.astype(np.float32) @ Wk.astype(np.float32)
    m4 = np.ascontiguousarray(
        M.astype(bf).reshape(IC, 128, 4, 256).transpose(2, 1, 0, 3)
    )
    wvT = np.ascontiguousarray(Wv.T.astype(bf).reshape(IC, 128, D).transpose(1, 0, 2))
    masks = [
        np.ascontiguousarray(
            _host_masks(par).reshape(NMSK, 128, 512).transpose(1, 0, 2)
        )
        for par in range(2)
    ]

    # per-parity column selection: core owns global k-chunks {2j+par}
    own_cols = [
        (128 * (2 * np.arange(LC)[:, None] + par) + np.arange(128)[None, :]).reshape(-1)
        for par in range(2)
    ]

    in_maps = []
    for c in range(8):
        b, par = c // 2, c % 2
        xTb = x[b].T[:, own_cols[par]].astype(bf)  # [D, S//2]
        # pack partition-major: [h, p, i, c]
        xpk = np.ascontiguousarray(
            xTb.reshape(IC, 128, 2, 512).transpose(2, 1, 0, 3)
        )
        in_maps.append(
            {
                "xT": xpk,
                "m4": m4,
                "wvT": wvT,
                "msk": masks[par],
            }
        )

    kwargs = {}
    if _trace:
        kwargs["trace"] = True
    res = run_bass_kernel_spmd(nc, in_maps, core_ids=list(range(8)), **kwargs)
    last_results = res

    # rows come back in gathered order; gath_row[q] = gathered position of q
    gath_row = np.empty(S, dtype=np.int64)
    gath_row[_gathered_q(np.arange(S))] = np.arange(S)

    out = np.empty((B, S, D), dtype=np.float32)
    for b in range(B):
        rA = res.results[2 * b]
        rB = res.results[2 * b + 1]
        num = rA["out_p"].astype(np.float32) + rB["out_p"].astype(np.float32)
        # den_out[r, qbg] is the partial denominator of gathered row
        # 128*qbg + r
        den = (
            rA["den_out"].T.reshape(S) + rB["den_out"].T.reshape(S)
        )
        out[b] = (num / den[:, None])[gath_row]
    return out
